# revision 1
# baseline (speedup 1.0000x reference)
"""Equivariant matmul kernel for Trainium2 (8 NeuronCores, Bass/Tile).

Problem (per edge e of E=800000):
    out[e,o,m] = (sum_i basis[e,o,i] * node_features[U[e],i,m]) * w[e,lo(o),m]
with D_IN=D_OUT=4, M=32, lo = [0,1,1,1].

Strategy (edge-parallel sharding, 100k edges/core, all math in fp32):
- Host prep per core shard (196 superblocks x 512 edges):
    * x_arr[s, 4b+i, 32g+m] = nf[U[e],i,m]  (gather + K-layout arrange)
    * wt[s, 32j+m, 128c+4b+o] = w_expanded  (for fused PSUM->SBUF multiply)
    * basis payloads packed for two on-device block-diag builders
- Device per superblock:
    * rhs = block-diagonal basis bd[4b+i, 128g+4b+o], built either by
      gpsimd.local_scatter (self-zeroing, int16-bitcast fp32) or by
      strided payload DMAs into persistent pre-zeroed SBUF (zeros are
      written once at kernel start and never again)
    * 16 TensorE matmuls (K=128, M=32, N=128, 4x col-tiled via
      tile_position) compute all 512 edges' 4x4 @ 4x32 products into
      one PSUM bank
    * one DVE tensor_mul applies the radial weights while draining
      PSUM->SBUF; bulk DMA out
- Host post: invert the layout permutation, concat shards.
"""

import contextlib
import ctypes
import sys
import types

import numpy as np

# ---------------------------------------------------------------- harness
# Workaround for walrus "Too many sync wait commands": this container's
# compiler accepts at most MAXW sem-waits per instruction; Tile emits more
# on the tail drain and occasionally mid-kernel. Split extras onto NOPs.
MAXW = 1


def _apply_tile_patch():
    import concourse.tile as tile_mod
    import concourse.mybir as mb
    from concourse.vector_clock import ScopedClock

    def _patched_drain_and_barrier(self, tick_clock, wait_clock):
        nc = self.nc
        drain_inst = nc.sync.drain()
        wait_clock.add_sem_waits(
            drain_inst.ins, ScopedClock({None: tick_clock.global_clock})
        )
        si = drain_inst.ins.sync_info
        if si is not None and len(si.on_wait) > 1:
            extra = list(si.on_wait[1:])
            si.on_wait = si.on_wait[:1]
            for w in extra:
                nop = nc.sync.nop(nofuse=True, hint="split_drain_wait")
                nop.ins.sync_info = mb.SyncInfo(on_wait=[w], on_update=[])
        nc.all_engine_barrier()
        assert self.sems is not None
        popped = nc._tile_sem_poison_stack.pop()
        assert popped is self._sem_poison
        nc.clear_and_free_semaphores(list(self.sems.allocated().values()))
        nc.all_engine_barrier()

    tile_mod.TileContext._drain_and_barrier = _patched_drain_and_barrier


_nop_counter = [0]


def _split_waits(nc, maxw=MAXW):
    import concourse.mybir as mb

    n_split = 0
    for fn in nc.m.functions:
        for blk in fn.blocks:
            insts = list(blk.instructions)
            out = []
            changed = False
            for inst in insts:
                si = getattr(inst, "sync_info", None)
                if si is not None and si.on_wait is not None and len(si.on_wait) > maxw:
                    extra = list(si.on_wait[:-maxw])
                    si.on_wait = list(si.on_wait[-maxw:])
                    for w in extra:
                        _nop_counter[0] += 1
                        nop = mb.InstNoOp(
                            name=f"waitsplit-{_nop_counter[0]}",
                            ins=[], outs=[], engine=inst.engine,
                        )
                        nop.sync_info = mb.SyncInfo(on_wait=[w], on_update=[])
                        out.append(nop)
                        n_split += 1
                    changed = True
                out.append(inst)
            if changed:
                blk.instructions = out
    return n_split


def _install_axon_ntff_hook():
    """Register the NTFF profile hook the agent image's antenv lacks, so
    run_bass_kernel_spmd(trace=True) can report HW exec time."""
    if "antenv.axon_hooks" in sys.modules:
        return
    so_path = "/opt/axon/libaxon_pjrt.so"
    holder = {}

    def _make_hook():
        try:
            lib = ctypes.CDLL(so_path)
        except OSError:
            return None
        if not hasattr(lib, "axon_start_nrt_profile"):
            return None
        lib.axon_start_nrt_profile.argtypes = [
            ctypes.POINTER(ctypes.c_int64), ctypes.c_size_t,
        ]
        lib.axon_start_nrt_profile.restype = ctypes.c_int64
        lib.axon_stop_nrt_profile.argtypes = [ctypes.c_char_p]
        lib.axon_stop_nrt_profile.restype = ctypes.c_int64

        @contextlib.contextmanager
        def _hook(output_dir, device_ids):
            import jax

            jax.devices()
            if device_ids:
                ids = (ctypes.c_int64 * len(device_ids))(*device_ids)
                rc = lib.axon_start_nrt_profile(ids, len(device_ids))
            else:
                rc = lib.axon_start_nrt_profile(None, 0)
            if rc != 0:
                raise RuntimeError(f"axon_start_nrt_profile rc={rc}")
            try:
                yield
            finally:
                n = lib.axon_stop_nrt_profile(str(output_dir).encode())
                if n < 0:
                    raise RuntimeError(f"axon_stop_nrt_profile rc={n}")

        return _hook

    mod = types.ModuleType("antenv.axon_hooks")
    mod.set_axon_ntff_profile_hook = lambda h: holder.__setitem__("h", h)
    mod.get_axon_ntff_profile_hook = lambda: holder.get("h")
    sys.modules["antenv.axon_hooks"] = mod
    try:
        import antenv

        antenv.axon_hooks = mod
    except ImportError:
        pass
    mod.set_axon_ntff_profile_hook(_make_hook())


# ---------------------------------------------------------------- config
N_CORES = 8
E = 800000
N_NODES = 50000
E_SHARD = E // N_CORES               # 100000
SB = 512                             # edges per superblock
NSB = (E_SHARD + SB - 1) // SB       # 196
E_PAD = NSB * SB                     # 100352
GROUPS = 16                          # 32-edge groups per superblock
BATCH = 4                            # superblocks per bd payload DMA batch
BD_W = 2048 * BATCH                  # fp32 columns per persistent bd tensor
LS_PERIOD = 8
LS_COUNT = 5                         # 5 of 8 SBs build bd via local_scatter

SB_MODES = ["ls" if (s % LS_PERIOD) < LS_COUNT else "dma" for s in range(NSB)]
DMA_SBS = [s for s in range(NSB) if SB_MODES[s] == "dma"]
LS_SBS = [s for s in range(NSB) if SB_MODES[s] == "ls"]
NDMA_BATCH = (len(DMA_SBS) + BATCH - 1) // BATCH

_CACHE = {}


# ---------------------------------------------------------------- program
def _build_program():
    import concourse.bass as bass
    import concourse.mybir as mb
    from concourse.tile import TileContext
    from concourse.library_overlay import lower_extended_insts
    from concourse.library_config import local_scatter as ls_lib

    nc = bass.Bass("TRN2", target_bir_lowering=False, debug=False,
                   num_devices=N_CORES)
    x_arr = nc.dram_tensor("x_arr", [NSB, 128, 512], mb.dt.float32,
                           kind="ExternalInput")
    wt = nc.dram_tensor("wt", [NSB, 128, 512], mb.dt.float32,
                        kind="ExternalInput")
    bsrc = nc.dram_tensor("bsrc", [NDMA_BATCH, 32, 4, BATCH, GROUPS, 4],
                          mb.dt.float32, kind="ExternalInput")
    lsd = nc.dram_tensor("lsd", [len(LS_SBS), 128, 128], mb.dt.int16,
                         kind="ExternalInput")
    lsi = nc.dram_tensor("lsi", [128, 32], mb.dt.int16, kind="ExternalInput")
    out_dev = nc.dram_tensor("out_dev", [NSB, 128, 512], mb.dt.float32,
                             kind="ExternalOutput")

    bds = [nc.alloc_sbuf_tensor(f"bd{k}", [128, BD_W], mb.dt.float32)
           for k in range(3)]

    dma_batch_of = {}
    for k, s in enumerate(DMA_SBS):
        dma_batch_of[s] = (k // BATCH, k % BATCH)
    ls_idx_of = {s: k for k, s in enumerate(LS_SBS)}

    with TileContext(nc) as tc:
        with (
            tc.tile_pool(name="xa", bufs=5) as x_pool,
            tc.tile_pool(name="wt", bufs=5) as wt_pool,
            tc.tile_pool(name="ld", bufs=3) as lsd_pool,
            tc.tile_pool(name="bl", bufs=12) as bdls_pool,
            tc.tile_pool(name="ou", bufs=4) as out_pool,
            tc.tile_pool(name="ps", bufs=3, space="PSUM") as psum_pool,
            tc.tile_pool(name="cs", bufs=1) as const_pool,
        ):
            nc.gpsimd.load_library(ls_lib)
            for bdt in bds:
                nc.gpsimd.memset(bdt.ap(), 0.0)
            lsit = const_pool.tile([128, 32], mb.dt.int16)
            nc.sync.dma_start(out=lsit[:], in_=lsi[:])

            issued_batches = set()
            pair = None
            for s in range(NSB):
                mode = SB_MODES[s]
                if mode == "dma":
                    bi, sp = dma_batch_of[s]
                    if bi not in issued_batches:
                        issued_batches.add(bi)
                        bd = bds[bi % 3]
                        for b in range(32):
                            dst = bass.AP(
                                bd.ap().tensor, 4 * b * BD_W + 4 * b,
                                [[BD_W, 4], [128, GROUPS * BATCH], [1, 4]])
                            eng = nc.sync if b % 2 == 0 else nc.scalar
                            eng.dma_start(out=dst, in_=bsrc[bi, b])
                    bd = bds[bi % 3]
                    rhs_base = ("p", bd, sp * 2048)
                else:
                    li = ls_idx_of[s]
                    lst = lsd_pool.tile([128, 128], mb.dt.int16)
                    nc.scalar.dma_start(out=lst[:], in_=lsd[li])
                    chunks = []
                    for c in range(4):
                        bdc = bdls_pool.tile([128, 1024], mb.dt.int16,
                                             tag="bdc")
                        nc.gpsimd.local_scatter(
                            out_ap=bdc[:],
                            data_ap=lst[:, c * 32:(c + 1) * 32],
                            idxs_ap=lsit[:], channels=128,
                            num_elems=1024, num_idxs=32,
                        )
                        chunks.append(bdc)
                    rhs_base = ("ls", chunks, 0)

                if s % 2 == 0:
                    xt2 = x_pool.tile([128, 1024], mb.dt.float32)
                    nc.sync.dma_start(
                        out=xt2[:].rearrange("p (s w) -> p s w", s=2),
                        in_=x_arr[s:s + 2].rearrange("s p w -> p s w"))
                    wt2 = wt_pool.tile([128, 1024], mb.dt.float32)
                    nc.scalar.dma_start(
                        out=wt2[:].rearrange("p (s w) -> p s w", s=2),
                        in_=wt[s:s + 2].rearrange("s p w -> p s w"))
                    pair = (xt2, wt2)
                xt2, wt2 = pair
                xt = xt2[:, (s % 2) * 512:(s % 2) * 512 + 512]
                wtile = wt2[:, (s % 2) * 512:(s % 2) * 512 + 512]

                psum = psum_pool.tile([128, 512], mb.dt.float32)
                for g in range(GROUPS):
                    c, j = g // 4, g % 4
                    if rhs_base[0] == "p":
                        bd, off = rhs_base[1], rhs_base[2]
                        rhs = bd.ap()[:, off + g * 128: off + g * 128 + 128]
                    else:
                        chunks = rhs_base[1]
                        rhs = chunks[c][:].bitcast(mb.dt.float32)[
                            :, 128 * j:128 * j + 128]
                    nc.tensor.matmul(
                        out=psum[32 * j:32 * j + 32, 128 * c:128 * c + 128],
                        lhsT=xt[:, 32 * g:32 * g + 32],
                        rhs=rhs,
                        start=True, stop=True,
                        tile_position=(0, 32 * j),
                    )
                otile = out_pool.tile([128, 512], mb.dt.float32)
                nc.vector.tensor_mul(out=otile[:], in0=psum[:], in1=wtile[:])
                nc.scalar.dma_start(out=out_dev[s], in_=otile[:])

    lower_extended_insts(nc)
    _split_waits(nc)
    return nc


# ---------------------------------------------------------------- host side
def _host_prep(basis, edge_weights, node_features, U):
    nf4 = np.ascontiguousarray(node_features, dtype=np.float32)

    p = np.arange(128)
    b_of_p = p // 4
    j = np.arange(4)
    o = np.arange(4)
    h = np.arange(2)
    lsi = (j[None, :, None, None] * 256 + 8 * b_of_p[:, None, None, None]
           + 2 * o[None, None, :, None] + h[None, None, None, :])
    lsi = lsi.reshape(128, 32).astype(np.int16)

    in_maps = []
    for core in range(N_CORES):
        lo = core * E_SHARD
        hi = lo + E_SHARD
        bshard = np.zeros((E_PAD, 4, 4), np.float32)
        bshard[:E_SHARD] = basis[lo:hi]
        w = np.zeros((E_PAD, 2, 32), np.float32)
        w[:E_SHARD] = edge_weights[lo:hi]
        u = np.zeros((E_PAD,), np.int64)
        u[:E_SHARD] = U[lo:hi]

        xg = nf4[u]                                     # [E_PAD, 4, 32]
        xa = xg.reshape(NSB, GROUPS, 32, 4, 32)         # [s,g,b,i,m]
        xa = xa.transpose(0, 2, 3, 1, 4)                # [s,b,i,g,m]
        x_arr = np.ascontiguousarray(xa.reshape(NSB, 128, 512), np.float32)

        bt = np.ascontiguousarray(bshard.transpose(0, 2, 1))  # [e, i, o]
        bts = bt.reshape(NSB, GROUPS, 32, 4, 4)         # [s,g,b,i,o]
        bsrc = np.zeros((NDMA_BATCH, 32, 4, BATCH, GROUPS, 4), np.float32)
        for k, s in enumerate(DMA_SBS):
            bi, sp = k // BATCH, k % BATCH
            bsrc[bi, :, :, sp, :, :] = bts[s].transpose(1, 2, 0, 3)
        lsd = np.zeros((len(LS_SBS), 128, 128), np.int16)
        bts_i16 = bts.view(np.int16).reshape(NSB, 4, 4, 32, 4, 8)
        for k, s in enumerate(LS_SBS):
            lsd[k] = bts_i16[s].transpose(2, 3, 0, 1, 4).reshape(128, 128)

        w_exp = w[:, [0, 1, 1, 1], :]
        wts = w_exp.reshape(NSB, 4, 4, 32, 4, 32)       # [s,c,j,b,o,m]
        wts = wts.transpose(0, 2, 5, 1, 3, 4)           # [s,j,m,c,b,o]
        wts = np.ascontiguousarray(wts.reshape(NSB, 128, 512), np.float32)

        in_maps.append({"x_arr": x_arr, "wt": wts, "bsrc": bsrc,
                        "lsd": lsd, "lsi": lsi})
    return in_maps


def _unshard(results):
    outs = []
    for core in range(N_CORES):
        od = results[core]["out_dev"]
        o6 = od.reshape(NSB, 4, 32, 4, 32, 4)           # [s,j,m,c,b,o]
        o6 = o6.transpose(0, 3, 1, 4, 5, 2)             # [s,c,j,b,o,m]
        outs.append(o6.reshape(E_PAD, 4, 32)[:E_SHARD])
    return np.concatenate(outs, axis=0)


# ---------------------------------------------------------------- entry
def kernel(basis, edge_weights, node_features, U, _trace=False):
    """Full inputs -> full output. Shards over 8 NeuronCores internally."""
    basis = np.asarray(basis, dtype=np.float32)
    edge_weights = np.asarray(edge_weights, dtype=np.float32)
    node_features = np.asarray(node_features, dtype=np.float32)
    U = np.asarray(U)

    _apply_tile_patch()
    _install_axon_ntff_hook()
    from concourse.bass_utils import run_bass_kernel_spmd

    if "nc" not in _CACHE:
        _CACHE["nc"] = _build_program()
    nc = _CACHE["nc"]

    in_maps = _host_prep(basis, edge_weights, node_features, U)
    res = run_bass_kernel_spmd(nc, in_maps, core_ids=list(range(N_CORES)),
                               trace=_trace)
    out = _unshard(res.results)
    if _trace:
        return out, res
    return out



# revision 5
# speedup vs baseline: 2.0030x; 2.0030x over previous
"""Equivariant matmul kernel for Trainium2 (8 NeuronCores, Bass/Tile).

Problem (per edge e of E=800000):
    out[e,o,m] = (sum_i basis[e,o,i] * node_features[U[e],i,m]) * w[e,lo(o),m]
with D_IN=D_OUT=4, M=32, lo = [0,1,1,1].

Strategy (edge-parallel sharding, 100k edges/core, fp16 data / fp32 PSUM):
- Host prep per core shard (196 superblocks x 512 edges, batched 7 SBs
  per DMA round -> 28 batches):
    * x_arr[nb, 4b+i, 512*sb + 32g+m] = nf[U[e],i,m]  (gather, fp16)
    * w_arr[nb, 32j+m, 256*sb + 64c+2b+l] = edge_weights (compact, fp16)
    * bsrc[nb, 4b+i, 448b-relative run (o,g,sb)] = basis^T payload (fp16)
- Device per batch:
    * one plain DMA each for x / w / payload / out; the payload lands in
      a persistent pre-zeroed block-diagonal tensor bd[4b+i, 448b+112o+
      7g+sb] via a partition-crossing strided AP (512B+ contiguous runs)
    * per superblock 16 fp16 TensorE matmuls (K=128, M=32, N=128,
      4x col-tiled) read bd through a strided AP; PSUM accumulates fp32
    * 2 DVE tensor_muls apply the radial weights (stride-0 broadcast
      expands the l=1 weight over o in {1,2,3}) draining PSUM->SBUF fp16
- Host post: invert the layout permutation, cast fp32, concat shards.
"""

import contextlib
import ctypes
import sys
import types

import numpy as np

# ---------------------------------------------------------------- harness
# Workaround for walrus "Too many sync wait commands": this container's
# compiler accepts at most MAXW sem-waits per instruction; Tile emits more
# on the tail drain and occasionally mid-kernel. Split extras onto NOPs.
MAXW = 1


def _apply_tile_patch():
    import concourse.tile as tile_mod
    import concourse.mybir as mb
    from concourse.vector_clock import ScopedClock

    def _patched_drain_and_barrier(self, tick_clock, wait_clock):
        nc = self.nc
        drain_inst = nc.sync.drain()
        wait_clock.add_sem_waits(
            drain_inst.ins, ScopedClock({None: tick_clock.global_clock})
        )
        si = drain_inst.ins.sync_info
        if si is not None and len(si.on_wait) > 1:
            extra = list(si.on_wait[1:])
            si.on_wait = si.on_wait[:1]
            for w in extra:
                nop = nc.sync.nop(nofuse=True, hint="split_drain_wait")
                nop.ins.sync_info = mb.SyncInfo(on_wait=[w], on_update=[])
        nc.all_engine_barrier()
        assert self.sems is not None
        popped = nc._tile_sem_poison_stack.pop()
        assert popped is self._sem_poison
        nc.clear_and_free_semaphores(list(self.sems.allocated().values()))
        nc.all_engine_barrier()

    tile_mod.TileContext._drain_and_barrier = _patched_drain_and_barrier


_nop_counter = [0]


def _split_waits(nc, maxw=MAXW):
    import concourse.mybir as mb

    n_split = 0
    for fn in nc.m.functions:
        for blk in fn.blocks:
            insts = list(blk.instructions)
            out = []
            changed = False
            for inst in insts:
                si = getattr(inst, "sync_info", None)
                if si is not None and si.on_wait is not None and len(si.on_wait) > maxw:
                    extra = list(si.on_wait[:-maxw])
                    si.on_wait = list(si.on_wait[-maxw:])
                    for w in extra:
                        _nop_counter[0] += 1
                        nop = mb.InstNoOp(
                            name=f"waitsplit-{_nop_counter[0]}",
                            ins=[], outs=[], engine=inst.engine,
                        )
                        nop.sync_info = mb.SyncInfo(on_wait=[w], on_update=[])
                        out.append(nop)
                        n_split += 1
                    changed = True
                out.append(inst)
            if changed:
                blk.instructions = out
    return n_split


def _install_axon_ntff_hook():
    """Register the NTFF profile hook the agent image's antenv lacks, so
    run_bass_kernel_spmd(trace=True) can report HW exec time."""
    if "antenv.axon_hooks" in sys.modules:
        return
    so_path = "/opt/axon/libaxon_pjrt.so"
    holder = {}

    def _make_hook():
        try:
            lib = ctypes.CDLL(so_path)
        except OSError:
            return None
        if not hasattr(lib, "axon_start_nrt_profile"):
            return None
        lib.axon_start_nrt_profile.argtypes = [
            ctypes.POINTER(ctypes.c_int64), ctypes.c_size_t,
        ]
        lib.axon_start_nrt_profile.restype = ctypes.c_int64
        lib.axon_stop_nrt_profile.argtypes = [ctypes.c_char_p]
        lib.axon_stop_nrt_profile.restype = ctypes.c_int64

        @contextlib.contextmanager
        def _hook(output_dir, device_ids):
            import jax

            jax.devices()
            if device_ids:
                ids = (ctypes.c_int64 * len(device_ids))(*device_ids)
                rc = lib.axon_start_nrt_profile(ids, len(device_ids))
            else:
                rc = lib.axon_start_nrt_profile(None, 0)
            if rc != 0:
                raise RuntimeError(f"axon_start_nrt_profile rc={rc}")
            try:
                yield
            finally:
                n = lib.axon_stop_nrt_profile(str(output_dir).encode())
                if n < 0:
                    raise RuntimeError(f"axon_stop_nrt_profile rc={n}")

        return _hook

    mod = types.ModuleType("antenv.axon_hooks")
    mod.set_axon_ntff_profile_hook = lambda h: holder.__setitem__("h", h)
    mod.get_axon_ntff_profile_hook = lambda: holder.get("h")
    sys.modules["antenv.axon_hooks"] = mod
    try:
        import antenv

        antenv.axon_hooks = mod
    except ImportError:
        pass
    mod.set_axon_ntff_profile_hook(_make_hook())


# ---------------------------------------------------------------- config
N_CORES = 8
E = 800000
N_NODES = 50000
E_SHARD = E // N_CORES               # 100000
SB = 512                             # edges per superblock
NSB = (E_SHARD + SB - 1) // SB       # 196
E_PAD = NSB * SB                     # 100352
GROUPS = 16                          # 32-edge groups per superblock
BATCH = 14                           # superblocks per DMA round
NB = NSB // BATCH                    # 14 batches
WBD = BATCH * 2048                   # 28672 fp16 cols per bd buffer
NBD = 2                              # bd double buffering
XW = BATCH * 512                     # 7168: x / out cols per batch
WW = BATCH * 256                     # 3584: weight cols per batch
PW = BATCH * 64                      # 896: payload cols per batch

_CACHE = {}


# ---------------------------------------------------------------- program
def _build_program(nbd=NBD, split=True):
    import concourse.bass as bass
    import concourse.mybir as mb
    from concourse.tile import TileContext

    nc = bass.Bass("TRN2", target_bir_lowering=False, debug=False,
                   num_devices=N_CORES)
    x_arr = nc.dram_tensor("x_arr", [NB, 128, XW], mb.dt.float16,
                           kind="ExternalInput")
    w_arr = nc.dram_tensor("w_arr", [NB, 128, WW], mb.dt.float16,
                           kind="ExternalInput")
    bsrc = nc.dram_tensor("bsrc", [NB, 128, PW], mb.dt.float16,
                          kind="ExternalInput")
    out_dev = nc.dram_tensor("out_dev", [NB, 128, XW], mb.dt.float16,
                             kind="ExternalOutput")

    # Persistent block-diagonal tensors; the zero slots are written once at
    # kernel start and never again (payload DMAs overwrite exactly the
    # nonzero runs each round).
    bds = [
        nc.alloc_sbuf_tensor(f"bd{k}", [128, WBD], mb.dt.float16)
        for k in range(nbd)
    ]

    with TileContext(nc) as tc:
        with (
            tc.tile_pool(name="xa", bufs=2) as x_pool,
            tc.tile_pool(name="wt", bufs=2) as wt_pool,
            tc.tile_pool(name="ou", bufs=2) as out_pool,
            tc.tile_pool(name="ps", bufs=4, space="PSUM") as psum_pool,
        ):
            for bdt in bds:
                nc.vector.memset(bdt.ap(), 0.0)

            for nb in range(NB):
                bdt = bds[nb % nbd]
                t = bdt.ap().tensor
                # per-block payload scatter: bd[4b:4b+4, PW*b : PW*(b+1)]
                for b in range(32):
                    dst = bass.AP(t, 4 * b * WBD + PW * b,
                                  [[WBD, 4], [1, PW]])
                    eng = nc.sync if b % 2 == 0 else nc.scalar
                    eng.dma_start(out=dst, in_=bsrc[nb, 4 * b:4 * b + 4, :])

                xt = x_pool.tile([128, XW], mb.dt.float16)
                nc.sync.dma_start(out=xt[:], in_=x_arr[nb])
                wt = wt_pool.tile([128, WW], mb.dt.float16)
                nc.scalar.dma_start(out=wt[:], in_=w_arr[nb])
                otile = out_pool.tile([128, XW], mb.dt.float16)

                for sb in range(BATCH):
                    psum = psum_pool.tile([128, 512], mb.dt.float32)
                    for g in range(GROUPS):
                        c, j = g // 4, g % 4
                        rhs = bass.AP(t, BATCH * g + sb,
                                      [[WBD, 128], [PW, 32], [BATCH * 16, 4]])
                        nc.tensor.matmul(
                            out=psum[32 * j:32 * j + 32,
                                     128 * c:128 * c + 128],
                            lhsT=xt[:, 512 * sb + 32 * g:512 * sb + 32 * g + 32],
                            rhs=rhs,
                            start=True, stop=True,
                            tile_position=(0, 32 * j),
                        )
                    # Radial-weight multiply while draining PSUM.
                    # psum[32j+m, 128c+4b+o] * w[32j+m, 64c+2b+lo(o)]
                    ps, ww, oo = psum[:], wt[:], otile[:]
                    o0_out = bass.AP(oo.tensor, oo.offset + 512 * sb,
                                     [oo.ap[0], [128, 4], [4, 32]])
                    o0_ps = bass.AP(ps.tensor, ps.offset,
                                    [ps.ap[0], [128, 4], [4, 32]])
                    o0_w = bass.AP(ww.tensor, ww.offset + 256 * sb,
                                   [ww.ap[0], [64, 4], [2, 32]])
                    nc.vector.tensor_mul(o0_out, o0_ps, o0_w)
                    o1_out = bass.AP(oo.tensor, oo.offset + 512 * sb + 1,
                                     [oo.ap[0], [128, 4], [4, 32], [1, 3]])
                    o1_ps = bass.AP(ps.tensor, ps.offset + 1,
                                    [ps.ap[0], [128, 4], [4, 32], [1, 3]])
                    o1_w = bass.AP(ww.tensor, ww.offset + 256 * sb + 1,
                                   [ww.ap[0], [64, 4], [2, 32], [0, 3]])
                    nc.vector.tensor_mul(o1_out, o1_ps, o1_w)

                nc.scalar.dma_start(out=out_dev[nb], in_=otile[:])

    if split:
        _split_waits(nc)
    return nc


# ---------------------------------------------------------------- host side
def _host_prep(basis, edge_weights, node_features, U):
    nf16 = np.ascontiguousarray(node_features).astype(np.float16)

    in_maps = []
    for core in range(N_CORES):
        lo = core * E_SHARD
        hi = lo + E_SHARD
        u = np.zeros((E_PAD,), np.int64)
        u[:E_SHARD] = U[lo:hi]

        # x_arr[nb, 4b+i, 512*sb + 32g+m], edge e = s*512 + g*32 + b,
        # s = nb*BATCH + sb
        xg = nf16[u]                                    # [E_PAD, 4, 32]
        xa = xg.reshape(NB, BATCH, GROUPS, 32, 4, 32)   # [nb,sb,g,b,i,m]
        xa = xa.transpose(0, 3, 4, 1, 2, 5)             # [nb,b,i,sb,g,m]
        x_arr = np.ascontiguousarray(
            xa.reshape(NB, 128, XW), np.float16)

        # w_arr[nb, 32j+m, 256*sb + 64c+2b+l]
        w = np.zeros((E_PAD, 2, 32), np.float16)
        w[:E_SHARD] = edge_weights[lo:hi].astype(np.float16)
        ws = w.reshape(NB, BATCH, 4, 4, 32, 2, 32)      # [nb,sb,c,j,b,l,m]
        ws = ws.transpose(0, 3, 6, 1, 2, 4, 5)          # [nb,j,m,sb,c,b,l]
        w_arr = np.ascontiguousarray(
            ws.reshape(NB, 128, WW), np.float16)

        # bsrc[nb, 4b+i, 112o + 7g + sb] = basis[e, o, i]
        b = np.zeros((E_PAD, 4, 4), np.float16)
        b[:E_SHARD] = basis[lo:hi].astype(np.float16)
        bs = b.reshape(NB, BATCH, GROUPS, 32, 4, 4)     # [nb,sb,g,b,o,i]
        bs = bs.transpose(0, 3, 5, 4, 2, 1)             # [nb,b,i,o,g,sb]
        bsrc = np.ascontiguousarray(
            bs.reshape(NB, 128, PW), np.float16)

        in_maps.append({"x_arr": x_arr, "w_arr": w_arr, "bsrc": bsrc})
    return in_maps


def _unshard(results):
    outs = []
    for core in range(N_CORES):
        od = results[core]["out_dev"]                   # [NB, 128, XW] fp16
        o7 = od.reshape(NB, 4, 32, BATCH, 4, 32, 4)     # [nb,j,m,sb,c,b,o]
        o7 = o7.transpose(0, 3, 4, 1, 5, 6, 2)          # [nb,sb,c,j,b,o,m]
        outs.append(
            o7.reshape(E_PAD, 4, 32)[:E_SHARD].astype(np.float32))
    return np.concatenate(outs, axis=0)


# ---------------------------------------------------------------- entry
def kernel(basis, edge_weights, node_features, U, _trace=False):
    """Full inputs -> full output. Shards over 8 NeuronCores internally."""
    basis = np.asarray(basis, dtype=np.float32)
    edge_weights = np.asarray(edge_weights, dtype=np.float32)
    node_features = np.asarray(node_features, dtype=np.float32)
    U = np.asarray(U)

    _apply_tile_patch()
    _install_axon_ntff_hook()
    from concourse.bass_utils import run_bass_kernel_spmd

    if "nc" not in _CACHE:
        _CACHE["nc"] = _build_program()
    nc = _CACHE["nc"]

    in_maps = _host_prep(basis, edge_weights, node_features, U)
    res = run_bass_kernel_spmd(nc, in_maps, core_ids=list(range(N_CORES)),
                               trace=_trace)
    out = _unshard(res.results)
    if _trace:
        return out, res
    return out


# revision 6
# speedup vs baseline: 2.3120x; 1.1542x over previous
"""Equivariant matmul kernel for Trainium2 (8 NeuronCores, Bass/Tile).

Problem (per edge e of E=800000):
    out[e,o,m] = (sum_i basis[e,o,i] * node_features[U[e],i,m]) * w[e,lo(o),m]
with D_IN=D_OUT=4, M=32, lo = [0,1,1,1].

Strategy (edge-parallel sharding, 100k edges/core, fp16 data / fp32 PSUM):
- Host prep per core shard (196 superblocks x 512 edges, batched 7 SBs
  per DMA round -> 28 batches):
    * x_arr[nb, 4b+i, 512*sb + 32g+m] = nf[U[e],i,m]  (gather, fp16)
    * w_arr[nb, 32j+m, 256*sb + 64c+2b+l] = edge_weights (compact, fp16)
    * bsrc[nb, 4b+i, 448b-relative run (o,g,sb)] = basis^T payload (fp16)
- Device per batch:
    * one plain DMA each for x / w / payload / out; the payload lands in
      a persistent pre-zeroed block-diagonal tensor bd[4b+i, 448b+112o+
      7g+sb] via a partition-crossing strided AP (512B+ contiguous runs)
    * per superblock 16 fp16 TensorE matmuls (K=128, M=32, N=128,
      4x col-tiled) read bd through a strided AP; PSUM accumulates fp32
    * 2 DVE tensor_muls apply the radial weights (stride-0 broadcast
      expands the l=1 weight over o in {1,2,3}) draining PSUM->SBUF fp16
- Host post: invert the layout permutation, cast fp32, concat shards.
"""

import contextlib
import ctypes
import sys
import types

import numpy as np

# ---------------------------------------------------------------- harness
# Workaround for walrus "Too many sync wait commands": this container's
# compiler accepts at most MAXW sem-waits per instruction; Tile emits more
# on the tail drain and occasionally mid-kernel. Split extras onto NOPs.
MAXW = 1


def _apply_tile_patch():
    import concourse.tile as tile_mod
    import concourse.mybir as mb
    from concourse.vector_clock import ScopedClock

    def _patched_drain_and_barrier(self, tick_clock, wait_clock):
        nc = self.nc
        drain_inst = nc.sync.drain()
        wait_clock.add_sem_waits(
            drain_inst.ins, ScopedClock({None: tick_clock.global_clock})
        )
        si = drain_inst.ins.sync_info
        if si is not None and len(si.on_wait) > 1:
            extra = list(si.on_wait[1:])
            si.on_wait = si.on_wait[:1]
            for w in extra:
                nop = nc.sync.nop(nofuse=True, hint="split_drain_wait")
                nop.ins.sync_info = mb.SyncInfo(on_wait=[w], on_update=[])
        nc.all_engine_barrier()
        assert self.sems is not None
        popped = nc._tile_sem_poison_stack.pop()
        assert popped is self._sem_poison
        nc.clear_and_free_semaphores(list(self.sems.allocated().values()))
        nc.all_engine_barrier()

    tile_mod.TileContext._drain_and_barrier = _patched_drain_and_barrier


_nop_counter = [0]


def _split_waits(nc, maxw=MAXW):
    import concourse.mybir as mb

    n_split = 0
    for fn in nc.m.functions:
        for blk in fn.blocks:
            insts = list(blk.instructions)
            out = []
            changed = False
            for inst in insts:
                si = getattr(inst, "sync_info", None)
                if si is not None and si.on_wait is not None and len(si.on_wait) > maxw:
                    extra = list(si.on_wait[:-maxw])
                    si.on_wait = list(si.on_wait[-maxw:])
                    for w in extra:
                        _nop_counter[0] += 1
                        nop = mb.InstNoOp(
                            name=f"waitsplit-{_nop_counter[0]}",
                            ins=[], outs=[], engine=inst.engine,
                        )
                        nop.sync_info = mb.SyncInfo(on_wait=[w], on_update=[])
                        out.append(nop)
                        n_split += 1
                    changed = True
                out.append(inst)
            if changed:
                blk.instructions = out
    return n_split


def _install_axon_ntff_hook():
    """Register the NTFF profile hook the agent image's antenv lacks, so
    run_bass_kernel_spmd(trace=True) can report HW exec time."""
    if "antenv.axon_hooks" in sys.modules:
        return
    so_path = "/opt/axon/libaxon_pjrt.so"
    holder = {}

    def _make_hook():
        try:
            lib = ctypes.CDLL(so_path)
        except OSError:
            return None
        if not hasattr(lib, "axon_start_nrt_profile"):
            return None
        lib.axon_start_nrt_profile.argtypes = [
            ctypes.POINTER(ctypes.c_int64), ctypes.c_size_t,
        ]
        lib.axon_start_nrt_profile.restype = ctypes.c_int64
        lib.axon_stop_nrt_profile.argtypes = [ctypes.c_char_p]
        lib.axon_stop_nrt_profile.restype = ctypes.c_int64

        @contextlib.contextmanager
        def _hook(output_dir, device_ids):
            import jax

            jax.devices()
            if device_ids:
                ids = (ctypes.c_int64 * len(device_ids))(*device_ids)
                rc = lib.axon_start_nrt_profile(ids, len(device_ids))
            else:
                rc = lib.axon_start_nrt_profile(None, 0)
            if rc != 0:
                raise RuntimeError(f"axon_start_nrt_profile rc={rc}")
            try:
                yield
            finally:
                n = lib.axon_stop_nrt_profile(str(output_dir).encode())
                if n < 0:
                    raise RuntimeError(f"axon_stop_nrt_profile rc={n}")

        return _hook

    mod = types.ModuleType("antenv.axon_hooks")
    mod.set_axon_ntff_profile_hook = lambda h: holder.__setitem__("h", h)
    mod.get_axon_ntff_profile_hook = lambda: holder.get("h")
    sys.modules["antenv.axon_hooks"] = mod
    try:
        import antenv

        antenv.axon_hooks = mod
    except ImportError:
        pass
    mod.set_axon_ntff_profile_hook(_make_hook())


# ---------------------------------------------------------------- config
N_CORES = 8
E = 800000
N_NODES = 50000
E_SHARD = E // N_CORES               # 100000
SB = 512                             # edges per superblock
NSB = (E_SHARD + SB - 1) // SB       # 196
E_PAD = NSB * SB                     # 100352
GROUPS = 16                          # 32-edge groups per superblock
BATCH = 7                            # superblocks per DMA round
NB = NSB // BATCH                    # 28 batches
NP = NB // 2                         # 14 batch pairs
WBD = BATCH * 2048                   # 14336 fp16 cols per bd region
XW = BATCH * 512                     # 3584: x / out cols per batch
WW = BATCH * 256                     # 1792: weight cols per batch
PW = BATCH * 64                      # 448: payload cols per batch

_CACHE = {}


# ---------------------------------------------------------------- program
def _build_program(split=True):
    import concourse.bass as bass
    import concourse.mybir as mb
    from concourse.tile import TileContext

    nc = bass.Bass("TRN2", target_bir_lowering=False, debug=False,
                   num_devices=N_CORES)
    x_arr = nc.dram_tensor("x_arr", [NB, 128, XW], mb.dt.float16,
                           kind="ExternalInput")
    w_arr = nc.dram_tensor("w_arr", [NB, 128, WW], mb.dt.float16,
                           kind="ExternalInput")
    bsrc = nc.dram_tensor("bsrc", [NP, 128, 2, PW], mb.dt.float16,
                          kind="ExternalInput")
    out_dev = nc.dram_tensor("out_dev", [NB, 128, XW], mb.dt.float16,
                             kind="ExternalOutput")

    # Two persistent block-diagonal tensors, two regions (= batches) each;
    # batch pair p lands in tensor p%2. The zero slots are written once at
    # kernel start and never again (payload DMAs overwrite exactly the
    # nonzero runs, one issue per 4-partition block covering both regions).
    W2 = 2 * WBD
    bds = [nc.alloc_sbuf_tensor(f"bd{k}", [128, W2], mb.dt.float16)
           for k in range(2)]

    with TileContext(nc) as tc:
        with (
            tc.tile_pool(name="xa", bufs=3) as x_pool,
            tc.tile_pool(name="wt", bufs=3) as wt_pool,
            tc.tile_pool(name="ou", bufs=3) as out_pool,
            tc.tile_pool(name="ps", bufs=4, space="PSUM") as psum_pool,
        ):
            # Stagger the zero-fills so pair 0's payload only waits on bd0:
            # DVE takes the low halves, GpSimd the high halves.
            nc.vector.memset(bds[0].ap()[:, :WBD], 0.0)
            nc.gpsimd.memset(bds[0].ap()[:, WBD:], 0.0)
            nc.vector.memset(bds[1].ap()[:, :WBD], 0.0)
            nc.gpsimd.memset(bds[1].ap()[:, WBD:], 0.0)

            for nb in range(NB):
                p, r = nb // 2, nb % 2
                bdt = bds[p % 2]
                t = bdt.ap().tensor
                if r == 0:
                    # pair payload: block b -> bd[4b:4b+4, both regions]
                    for b in range(32):
                        dst = bass.AP(t, 4 * b * W2 + PW * b,
                                      [[W2, 4], [WBD, 2], [1, PW]])
                        eng = nc.sync if b % 2 == 0 else nc.scalar
                        eng.dma_start(out=dst, in_=bsrc[p, 4 * b:4 * b + 4])

                xt = x_pool.tile([128, XW], mb.dt.float16)
                nc.sync.dma_start(out=xt[:], in_=x_arr[nb])
                wt = wt_pool.tile([128, WW], mb.dt.float16)
                nc.scalar.dma_start(out=wt[:], in_=w_arr[nb])
                otile = out_pool.tile([128, XW], mb.dt.float16)

                for sb in range(BATCH):
                    psum = psum_pool.tile([128, 512], mb.dt.float32)
                    for g in range(GROUPS):
                        c, j = g // 4, g % 4
                        rhs = bass.AP(t, r * WBD + BATCH * g + sb,
                                      [[W2, 128], [PW, 32], [BATCH * 16, 4]])
                        nc.tensor.matmul(
                            out=psum[32 * j:32 * j + 32,
                                     128 * c:128 * c + 128],
                            lhsT=xt[:, 512 * sb + 32 * g:512 * sb + 32 * g + 32],
                            rhs=rhs,
                            start=True, stop=True,
                            tile_position=(0, 32 * j),
                        )
                    # Radial-weight multiply while draining PSUM.
                    # psum[32j+m, 128c+4b+o] * w[32j+m, 64c+2b+lo(o)]
                    ps, ww, oo = psum[:], wt[:], otile[:]
                    o0_out = bass.AP(oo.tensor, oo.offset + 512 * sb,
                                     [oo.ap[0], [128, 4], [4, 32]])
                    o0_ps = bass.AP(ps.tensor, ps.offset,
                                    [ps.ap[0], [128, 4], [4, 32]])
                    o0_w = bass.AP(ww.tensor, ww.offset + 256 * sb,
                                   [ww.ap[0], [64, 4], [2, 32]])
                    nc.vector.tensor_mul(o0_out, o0_ps, o0_w)
                    o1_out = bass.AP(oo.tensor, oo.offset + 512 * sb + 1,
                                     [oo.ap[0], [128, 4], [4, 32], [1, 3]])
                    o1_ps = bass.AP(ps.tensor, ps.offset + 1,
                                    [ps.ap[0], [128, 4], [4, 32], [1, 3]])
                    o1_w = bass.AP(ww.tensor, ww.offset + 256 * sb + 1,
                                   [ww.ap[0], [64, 4], [2, 32], [0, 3]])
                    nc.vector.tensor_mul(o1_out, o1_ps, o1_w)

                nc.scalar.dma_start(out=out_dev[nb], in_=otile[:])

    if split:
        _split_waits(nc)
    return nc


# ---------------------------------------------------------------- host side
def _host_prep(basis, edge_weights, node_features, U):
    nf16 = np.ascontiguousarray(node_features).astype(np.float16)

    in_maps = []
    for core in range(N_CORES):
        lo = core * E_SHARD
        hi = lo + E_SHARD
        u = np.zeros((E_PAD,), np.int64)
        u[:E_SHARD] = U[lo:hi]

        # x_arr[nb, 4b+i, 512*sb + 32g+m], edge e = s*512 + g*32 + b,
        # s = nb*BATCH + sb
        xg = nf16[u]                                    # [E_PAD, 4, 32]
        xa = xg.reshape(NB, BATCH, GROUPS, 32, 4, 32)   # [nb,sb,g,b,i,m]
        xa = xa.transpose(0, 3, 4, 1, 2, 5)             # [nb,b,i,sb,g,m]
        x_arr = np.ascontiguousarray(
            xa.reshape(NB, 128, XW), np.float16)

        # w_arr[nb, 32j+m, 256*sb + 64c+2b+l]
        w = np.zeros((E_PAD, 2, 32), np.float16)
        w[:E_SHARD] = edge_weights[lo:hi].astype(np.float16)
        ws = w.reshape(NB, BATCH, 4, 4, 32, 2, 32)      # [nb,sb,c,j,b,l,m]
        ws = ws.transpose(0, 3, 6, 1, 2, 4, 5)          # [nb,j,m,sb,c,b,l]
        w_arr = np.ascontiguousarray(
            ws.reshape(NB, 128, WW), np.float16)

        # bsrc[p, 4b+i, r, 112o + 7g + sb] = basis[e, o, i], nb = 2p + r
        b = np.zeros((E_PAD, 4, 4), np.float16)
        b[:E_SHARD] = basis[lo:hi].astype(np.float16)
        bs = b.reshape(NP, 2, BATCH, GROUPS, 32, 4, 4)  # [p,r,sb,g,b,o,i]
        bs = bs.transpose(0, 4, 6, 1, 5, 3, 2)          # [p,b,i,r,o,g,sb]
        bsrc = np.ascontiguousarray(
            bs.reshape(NP, 128, 2, PW), np.float16)

        in_maps.append({"x_arr": x_arr, "w_arr": w_arr, "bsrc": bsrc})
    return in_maps


def _unshard(results):
    outs = []
    for core in range(N_CORES):
        od = results[core]["out_dev"]                   # [NB, 128, XW] fp16
        o7 = od.reshape(NB, 4, 32, BATCH, 4, 32, 4)     # [nb,j,m,sb,c,b,o]
        o7 = o7.transpose(0, 3, 4, 1, 5, 6, 2)          # [nb,sb,c,j,b,o,m]
        outs.append(
            o7.reshape(E_PAD, 4, 32)[:E_SHARD].astype(np.float32))
    return np.concatenate(outs, axis=0)


# ---------------------------------------------------------------- entry
def kernel(basis, edge_weights, node_features, U, _trace=False):
    """Full inputs -> full output. Shards over 8 NeuronCores internally."""
    basis = np.asarray(basis, dtype=np.float32)
    edge_weights = np.asarray(edge_weights, dtype=np.float32)
    node_features = np.asarray(node_features, dtype=np.float32)
    U = np.asarray(U)

    _apply_tile_patch()
    _install_axon_ntff_hook()
    from concourse.bass_utils import run_bass_kernel_spmd

    if "nc" not in _CACHE:
        _CACHE["nc"] = _build_program()
    nc = _CACHE["nc"]

    in_maps = _host_prep(basis, edge_weights, node_features, U)
    res = run_bass_kernel_spmd(nc, in_maps, core_ids=list(range(N_CORES)),
                               trace=_trace)
    out = _unshard(res.results)
    if _trace:
        return out, res
    return out
